# revision 24
# baseline (speedup 1.0000x reference)
"""Trainium2 Bass kernel for nn_Criterion_37984690765901 (v2).

Loss = L_t + lam_e * Loss_e + lam_od * (L_zt + L_zs)
  L_t    = mean_r( lse(y_r) - y[r, target_r] )
  Loss_e = mean_r( lse(s_r) - (sum_j e^{s_rj} s_rj)/sum_j e^{s_rj} )
  L_z    = mean_r( RD_r/S_r - ln S_r + ln PS_r )
           std = exp(0.5 ls), se = std*eps, enc = se + mean,
           e = exp(enc), d = enc - prior, ed = e*d, pe = exp(prior),
           S = sum_d e, RD = sum_d ed, PS = sum_d pe.

v2 design (from measured TRN2 engine rates):
- Pure data parallel, 8192 rows/core.  Big tensors are shipped
  TRANSPOSED ([D=128 partitions, rows free], host-packed) so the three
  per-row reductions run on the idle PE as stationary-data matmuls
  (data chunk [128,128] as weights, ones as moving; ~28ns/chunk).
- dtypes: ls/eps fp8e4m3, mean/prior bf16 (12.4MB/core vs 33.4 f32).
  fp8 operands force DVE to 1x, so fp8 is only read by ACT (dtype-
  independent) and the Pool engine (software, dtype-independent).
- Engine split per chunk: ACT: std=exp(.5*ls8), e=exp(enc).
  Pool: se=std*eps8.  DVE: pe via int16 exp bit-trick (TENSOR_SCALAR
  runs 4x for 2-byte dtypes), enc/d/ed as bf16 TTs (2x).
  PE: per-row sums of e, ed, pe into PSUM f32.
- Host: pack/cast/transpose only, plus f64 sum of per-row partials
  (same contract as v1: one-hot/pick gather is indexing prep).

Accuracy (host-simulated, bit-exact TS trick formula): rel err ~3e-4
vs f64 reference (tolerance 2e-2).
"""

import os
import numpy as np
import ml_dtypes

NCORES = 8
B, D, C, SC = 65536, 128, 10, 2
LAMBDA_E, LAMBDA_OD = 0.1, 0.036
GAMMA_E, GAMMA_OD = 2.0, 2.0
STEP_SIZE = 1000.0

P = 128
RPC = B // NCORES            # 8192 rows per core
RC = 2048                    # rows per compute chunk
SLAB = 4096                  # rows per DMA slab (2 compute chunks)
NCH = RPC // RC              # 4 chunks per branch
NSTEPS = 2 * NCH             # interleaved t/s steps
CPC = RC // P                # 16 psum cols per chunk
NCOL = RPC // P              # 64 psum cols per quantity
YF = RPC * C // P            # 640
SF = RPC * SC // P           # 128

# exp bit trick: bf16 bits of exp(x) ~= round(x*K1 + K2) as int16
K1 = 128.0 * 1.4426950408889634
K2 = 128.0 * (127.0 - 0.043)

BR = ["t", "s"]

_CACHED_NC = None
LAST_EXEC_NS = None


def _build_nc():
    import concourse.bass as bass
    import concourse.tile as tile
    from concourse import mybir
    from contextlib import ExitStack

    f32 = mybir.dt.float32
    bf16 = mybir.dt.bfloat16
    i16 = mybir.dt.int16
    fp8 = mybir.dt.float8e4
    Exp = mybir.ActivationFunctionType.Exp
    Ln = mybir.ActivationFunctionType.Ln
    add = mybir.AluOpType.add
    sub = mybir.AluOpType.subtract
    mult = mybir.AluOpType.mult
    X = mybir.AxisListType.X

    nc = bass.Bass("TRN2", debug=False)

    ins = {}
    for bn in BR:
        ins[f"ls_{bn}"] = nc.dram_tensor(f"ls_{bn}", [P, RPC], fp8,
                                         kind="ExternalInput").ap()
        ins[f"eps_{bn}"] = nc.dram_tensor(f"eps_{bn}", [P, RPC], bf16,
                                          kind="ExternalInput").ap()
        ins[f"mean_{bn}"] = nc.dram_tensor(f"mean_{bn}", [P, RPC], bf16,
                                           kind="ExternalInput").ap()
        ins[f"prior_{bn}"] = nc.dram_tensor(f"prior_{bn}", [P, RPC], bf16,
                                            kind="ExternalInput").ap()
    ins["y16"] = nc.dram_tensor("y16", [P, YF], bf16, kind="ExternalInput").ap()
    ins["ypick"] = nc.dram_tensor("ypick", [P, NCOL], f32,
                                  kind="ExternalInput").ap()
    ins["sz16"] = nc.dram_tensor("sz16", [P, SF], bf16,
                                 kind="ExternalInput").ap()
    out_d = nc.dram_tensor("out", [P, 4 * NCOL], f32, kind="ExternalOutput").ap()

    with tile.TileContext(nc) as tc, ExitStack() as ctx:
        io = ctx.enter_context(tc.tile_pool(name="io", bufs=1))
        st = ctx.enter_context(tc.tile_pool(name="st", bufs=1))
        ps = ctx.enter_context(tc.tile_pool(name="ps", bufs=1, space="PSUM"))

        out_sb = st.tile([P, 4 * NCOL], f32, tag="out")
        ones_b = st.tile([P, 1], bf16, tag="ones_b")
        nc.vector.memset(ones_b[:], 1.0)

        # PSUM accumulators: [quantity][branch] -> [P, NCOL] f32
        ps_S = [ps.tile([P, NCOL], f32, tag=f"psS{b}", name=f"psS{b}")
                for b in range(2)]
        ps_RD = [ps.tile([P, NCOL], f32, tag=f"psRD{b}", name=f"psRD{b}")
                 for b in range(2)]
        ps_PS = [ps.tile([P, NCOL], f32, tag=f"psPS{b}", name=f"psPS{b}")
                 for b in range(2)]

        # ---------------- small paths first (pipeline warm-up) -------------
        y_t = st.tile([P, YF], bf16, tag="y16")
        nc.sync.dma_start(y_t[:], ins["y16"])
        yp_t = st.tile([P, NCOL], f32, tag="ypick")
        nc.sync.dma_start(yp_t[:], ins["ypick"])
        sz_t = st.tile([P, SF], bf16, tag="sz16")
        nc.sync.dma_start(sz_t[:], ins["sz16"])

        # cross entropy: ce_row = ln(sum_c exp(y)) - y_pick
        ey_t = st.tile([P, YF], bf16, tag="ey")
        nc.scalar.activation(ey_t[:], y_t[:], Exp)
        sy_t = st.tile([P, NCOL], f32, tag="sy")
        nc.vector.tensor_reduce(
            sy_t[:], ey_t[:].rearrange("p (g c) -> p g c", c=C), X, add)
        lse_t = st.tile([P, NCOL], f32, tag="lse")
        nc.scalar.activation(lse_t[:], sy_t[:], Ln)
        nc.vector.tensor_tensor(
            out_sb[:, bass.ts(2, NCOL)], lse_t[:], yp_t[:], sub)
        nc.sync.dma_start(out_d[:, bass.ts(2, NCOL)], out_sb[:, bass.ts(2, NCOL)])

        # entropy of softmax(s_zt): ln(ssum) - (sum e*x)/ssum
        esz_t = st.tile([P, SF], bf16, tag="esz")
        nc.scalar.activation(esz_t[:], sz_t[:], Exp)
        ssum_t = st.tile([P, NCOL], f32, tag="ssum")
        nc.vector.tensor_reduce(
            ssum_t[:], esz_t[:].rearrange("p (g c) -> p g c", c=SC), X, add)
        exs_t = st.tile([P, SF], bf16, tag="exs")
        nc.vector.tensor_tensor(exs_t[:], esz_t[:], sz_t[:], mult)
        dsum_t = st.tile([P, NCOL], f32, tag="dsum")
        nc.vector.tensor_reduce(
            dsum_t[:], exs_t[:].rearrange("p (g c) -> p g c", c=SC), X, add)
        rss_t = st.tile([P, NCOL], f32, tag="rss")
        nc.vector.reciprocal(rss_t[:], ssum_t[:])
        t2_t = st.tile([P, NCOL], f32, tag="t2")
        nc.vector.tensor_tensor(t2_t[:], dsum_t[:], rss_t[:], mult)
        lss_t = st.tile([P, NCOL], f32, tag="lss")
        nc.scalar.activation(lss_t[:], ssum_t[:], Ln)
        nc.vector.tensor_tensor(
            out_sb[:, bass.ts(3, NCOL)], lss_t[:], t2_t[:], sub)
        nc.sync.dma_start(out_d[:, bass.ts(3, NCOL)], out_sb[:, bass.ts(3, NCOL)])

        # ---------------- upfront input DMAs -------------------------------
        # all big tensors fit in SBUF (112KB/partition).  Stream them in
        # chunk-granular DMAs ordered by first use (both branches' chunk 0
        # first) so the pipeline starts ~15us earlier than a slab stream.
        full = {}
        for tname, dt_ in (("ls", fp8), ("prior", bf16),
                           ("eps", bf16), ("mean", bf16)):
            for bn in BR:
                full[(bn, tname)] = st.tile(
                    [P, RPC], dt_, tag=f"{tname}_{bn}", name=f"{tname}_{bn}")
        # single SP (sync) HWDGE queue; half-tensor DMAs (8-16KB
        # descriptors, few queue handoffs), ordered by first use.
        # (Measured dead ends: gpsimd SWDGE as a second queue runs at
        # half rate; scalar-engine HWDGE as a second queue is slower
        # still; per-chunk DMAs pay ~0.65us handoff each.)
        H = RPC // 2
        for lo, hi in ((0, H), (H, RPC)):
            for bn in BR:
                for tname in ("ls", "prior", "eps", "mean"):
                    nc.sync.dma_start(
                        full[(bn, tname)][:, lo:hi],
                        ins[f"{tname}_{bn}"][:, lo:hi])

        def chunk(bn, tname, c):
            return full[(bn, tname)][:, c * RC:(c + 1) * RC]

        # ---------------- main pipeline ------------------------------------
        # step s: branch b = s % 2, chunk c = s // 2, col range c*CPC..
        # Engines: ACT: std, e.  DVE: pe bit-trick, se, enc, d, ed.
        # PE: per-row sums.  (Pool deliberately idle: its software TTs
        # stall concurrent DVE ops ~4x.)
        state = {}

        def stage1(s):
            # ACT std; pe = exp(prior): ACT for branch t, DVE bit-trick for
            # branch s (load balance)
            b, c = s % 2, s // 2
            bn = BR[b]
            t = state[s] = {}
            t["std"] = io.tile([P, RC], bf16, tag="std", name=f"std{s}", bufs=3)
            nc.scalar.activation(t["std"][:], chunk(bn, "ls", c), Exp, scale=0.5)
            t["pe"] = io.tile([P, RC], bf16, tag="pe", name=f"pe{s}", bufs=3)
            if b == 0:
                nc.scalar.activation(t["pe"][:], chunk(bn, "prior", c), Exp)
            else:
                nc.vector.tensor_scalar(
                    t["pe"][:].bitcast(i16), chunk(bn, "prior", c), K1, K2,
                    mult, add)

        def stage2(s):
            # DVE se = std * eps; PE: per-row sums of pe into PSUM
            b, c = s % 2, s // 2
            bn = BR[b]
            t = state[s]
            t["se"] = io.tile([P, RC], bf16, tag="se", name=f"se{s}", bufs=3)
            nc.vector.tensor_tensor(
                t["se"][:], t["std"][:], chunk(bn, "eps", c), mult)
            for k in range(CPC):
                nc.tensor.matmul(
                    ps_PS[b][:, c * CPC + k:c * CPC + k + 1],
                    t["pe"][:, k * P:(k + 1) * P], ones_b[:])

        def stage3(s):
            # DVE enc, d; ACT e
            b, c = s % 2, s // 2
            bn = BR[b]
            t = state[s]
            t["enc"] = io.tile([P, RC], bf16, tag="enc", name=f"enc{s}", bufs=3)
            nc.vector.tensor_tensor(
                t["enc"][:], t["se"][:], chunk(bn, "mean", c), add)
            t["d"] = io.tile([P, RC], bf16, tag="d", name=f"d{s}", bufs=3)
            nc.vector.tensor_tensor(
                t["d"][:], t["enc"][:], chunk(bn, "prior", c), sub)
            t["e"] = io.tile([P, RC], bf16, tag="e", name=f"e{s}", bufs=3)
            nc.scalar.activation(t["e"][:], t["enc"][:], Exp)

        def stage4(s):
            # DVE ed; PE: sums of e and ed
            b, c = s % 2, s // 2
            t = state.pop(s)
            t["ed"] = io.tile([P, RC], bf16, tag="ed", name=f"ed{s}", bufs=3)
            nc.vector.tensor_tensor(t["ed"][:], t["e"][:], t["d"][:], mult)
            for k in range(CPC):
                nc.tensor.matmul(
                    ps_S[b][:, c * CPC + k:c * CPC + k + 1],
                    t["e"][:, k * P:(k + 1) * P], ones_b[:])
            for k in range(CPC):
                nc.tensor.matmul(
                    ps_RD[b][:, c * CPC + k:c * CPC + k + 1],
                    t["ed"][:, k * P:(k + 1) * P], ones_b[:])

        for i in range(NSTEPS + 3):
            if i < NSTEPS:
                stage1(i)
            if 1 <= i and i - 1 < NSTEPS:
                stage2(i - 1)
            if 2 <= i and i - 2 < NSTEPS:
                stage3(i - 2)
            if 3 <= i and i - 3 < NSTEPS:
                stage4(i - 3)

        # ---------------- per-branch tails ---------------------------------
        for b in range(2):
            rs_t = st.tile([P, NCOL], f32, tag=f"rs{b}")
            nc.vector.reciprocal(rs_t[:], ps_S[b][:])
            term_t = st.tile([P, NCOL], f32, tag=f"term{b}")
            nc.vector.tensor_tensor(term_t[:], ps_RD[b][:], rs_t[:], mult)
            lnS_t = st.tile([P, NCOL], f32, tag=f"lnS{b}")
            nc.scalar.activation(lnS_t[:], ps_S[b][:], Ln)
            lnPS_t = st.tile([P, NCOL], f32, tag=f"lnPS{b}")
            nc.scalar.activation(lnPS_t[:], ps_PS[b][:], Ln)
            tmp_t = st.tile([P, NCOL], f32, tag=f"tmp{b}")
            nc.vector.tensor_tensor(tmp_t[:], term_t[:], lnS_t[:], sub)
            nc.vector.tensor_tensor(
                out_sb[:, bass.ts(b, NCOL)], tmp_t[:], lnPS_t[:], add)
            nc.sync.dma_start(
                out_d[:, bass.ts(b, NCOL)], out_sb[:, bass.ts(b, NCOL)])

    return nc


def _split_multi_waits(nc):
    """walrus's codegen allows a single embedded sync-wait per compute
    instruction; Tile sometimes emits two.  Hoist all-but-one wait into
    standalone EventSemaphore instructions on the same engine."""
    import json

    orig = nc.to_json_bytes

    def patched():
        bj = json.loads(orig())
        for fn in bj["functions"]:
            for blk in fn["blocks"]:
                new = []
                for inst in blk["instructions"]:
                    si = inst.get("sync_info") or {}
                    waits = si.get("on_wait") or []
                    if len(waits) > 1 and inst.get("opcode") != "EventSemaphore":
                        for i, w in enumerate(waits[:-1]):
                            new.append({
                                "debug": inst.get("debug"),
                                "engine": inst["engine"],
                                "ins": [],
                                "name": f"{inst['name']}-sw{i}",
                                "opcode": "EventSemaphore",
                                "outs": [],
                                "sync_info": {"on_update": [], "on_wait": [w]},
                            })
                        si["on_wait"] = [waits[-1]]
                    new.append(inst)
                blk["instructions"] = new
        return json.dumps(bj).encode()

    nc.to_json_bytes = patched
    return nc


def get_nc():
    global _CACHED_NC
    if _CACHED_NC is None:
        _CACHED_NC = _split_multi_waits(_build_nc())
    return _CACHED_NC


def make_in_maps(inputs):
    """Host-side sharding: slice rows, cast dtypes, transpose to
    [D, rows] for the big tensors, gather y_pick."""
    f8 = ml_dtypes.float8_e4m3
    bf = ml_dtypes.bfloat16
    arr = {k: np.asarray(v) for k, v in inputs.items()}
    target = np.asarray(arr["target"]).astype(np.int64).reshape(B)
    y32 = np.asarray(arr["y_zt"], np.float32)
    ypick_full = y32[np.arange(B), target]

    srcs = {
        "t": ("log_std_t", "eps_t", "mean_t", "eps_prior_t"),
        "s": ("log_std_s", "eps_s", "mean_s", "eps_prior_s"),
    }
    in_maps = []
    for cidx in range(NCORES):
        sl = slice(cidx * RPC, (cidx + 1) * RPC)
        m = {}
        for bn, (ls_k, eps_k, mean_k, prior_k) in srcs.items():
            m[f"ls_{bn}"] = np.ascontiguousarray(
                np.asarray(arr[ls_k][sl], np.float32).astype(f8).T)
            m[f"eps_{bn}"] = np.ascontiguousarray(
                np.asarray(arr[eps_k][sl], np.float32).astype(bf).T)
            m[f"mean_{bn}"] = np.ascontiguousarray(
                np.asarray(arr[mean_k][sl], np.float32).astype(bf).T)
            m[f"prior_{bn}"] = np.ascontiguousarray(
                np.asarray(arr[prior_k][sl], np.float32).astype(bf).T)
        m["y16"] = np.ascontiguousarray(
            y32[sl].astype(bf).reshape(P, YF))
        m["ypick"] = np.ascontiguousarray(
            ypick_full[sl].astype(np.float32).reshape(P, NCOL))
        m["sz16"] = np.ascontiguousarray(
            np.asarray(arr["s_zt"][sl], np.float32).astype(bf).reshape(P, SF))
        in_maps.append(m)
    return in_maps


def combine(outs, current_step):
    """Host-side unshard: f64 reduce of per-row partials -> final scalar."""
    tot = np.zeros(4, dtype=np.float64)
    for o in outs:
        o = np.asarray(o, np.float64).reshape(P, 4, NCOL)
        tot += o.sum(axis=(0, 2))
    L_zt, L_zs, L_t, Loss_e = tot / B
    frac = float(current_step) / STEP_SIZE
    lam_e = LAMBDA_E * GAMMA_E ** frac
    lam_od = LAMBDA_OD * GAMMA_OD ** frac
    val = L_t + lam_e * Loss_e + lam_od * (L_zt + L_zs)
    return np.array(val, dtype=np.float32)


def _install_ntff_hook():
    """Best-effort: register the axon NTFF profiling hook so trace=True
    yields exec_time_ns."""
    try:
        import sys, types
        import antenv
        if "antenv.axon_hooks" in sys.modules:
            return True
        sys.path.insert(0, "/root/.axon_site/trn_agent_boot")
        import trn_boot
        mod = types.ModuleType("antenv.axon_hooks")
        _h = {}
        mod.set_axon_ntff_profile_hook = lambda h: _h.__setitem__("h", h)
        mod.get_axon_ntff_profile_hook = lambda: _h.get("h")
        sys.modules["antenv.axon_hooks"] = mod
        antenv.axon_hooks = mod
        mod.set_axon_ntff_profile_hook(
            trn_boot._ntff_profile_via_ctypes("/opt/axon/libaxon_pjrt.so")
        )
        import concourse.bass_utils as bu
        bu.upload_artifacts = lambda tmpdir: str(tmpdir)
        return True
    except Exception:
        return False


def kernel(**inputs):
    global LAST_EXEC_NS
    from concourse.bass_utils import run_bass_kernel_spmd

    trace = os.environ.get("BASS_KERNEL_TRACE", "0") == "1"
    if trace:
        trace = _install_ntff_hook()

    nc = get_nc()
    in_maps = make_in_maps(inputs)
    res = run_bass_kernel_spmd(
        nc, in_maps, list(range(NCORES)), trace=trace
    )
    LAST_EXEC_NS = res.exec_time_ns
    outs = [r["out"] for r in res.results]
    cs = inputs.get("current_step", 500)
    return combine(outs, int(np.asarray(cs)))


# revision 25
# speedup vs baseline: 1.0690x; 1.0690x over previous
"""Trainium2 Bass kernel for nn_Criterion_37984690765901 (v2).

Loss = L_t + lam_e * Loss_e + lam_od * (L_zt + L_zs)
  L_t    = mean_r( lse(y_r) - y[r, target_r] )
  Loss_e = mean_r( lse(s_r) - (sum_j e^{s_rj} s_rj)/sum_j e^{s_rj} )
  L_z    = mean_r( RD_r/S_r - ln S_r + ln PS_r )
           std = exp(0.5 ls), se = std*eps, enc = se + mean,
           e = exp(enc), d = enc - prior, ed = e*d, pe = exp(prior),
           S = sum_d e, RD = sum_d ed, PS = sum_d pe.

v2 design (from measured TRN2 engine rates):
- Pure data parallel, 8192 rows/core.  Big tensors are shipped
  TRANSPOSED ([D=128 partitions, rows free], host-packed) so the three
  per-row reductions run on the idle PE as stationary-data matmuls
  (data chunk [128,128] as weights, ones as moving; ~28ns/chunk).
- dtypes: ls/eps fp8e4m3, mean/prior bf16 (12.4MB/core vs 33.4 f32).
  fp8 operands force DVE to 1x, so fp8 is only read by ACT (dtype-
  independent) and the Pool engine (software, dtype-independent).
- Engine split per chunk: ACT: std=exp(.5*ls8), e=exp(enc).
  Pool: se=std*eps8.  DVE: pe via int16 exp bit-trick (TENSOR_SCALAR
  runs 4x for 2-byte dtypes), enc/d/ed as bf16 TTs (2x).
  PE: per-row sums of e, ed, pe into PSUM f32.
- Host: pack/cast/transpose only, plus f64 sum of per-row partials
  (same contract as v1: one-hot/pick gather is indexing prep).

Accuracy (host-simulated, bit-exact TS trick formula): rel err ~3e-4
vs f64 reference (tolerance 2e-2).
"""

import os
import numpy as np
import ml_dtypes

NCORES = 8
B, D, C, SC = 65536, 128, 10, 2
LAMBDA_E, LAMBDA_OD = 0.1, 0.036
GAMMA_E, GAMMA_OD = 2.0, 2.0
STEP_SIZE = 1000.0

P = 128
RPC = B // NCORES            # 8192 rows per core
RC = 2048                    # rows per compute chunk
SLAB = 4096                  # rows per DMA slab (2 compute chunks)
NCH = RPC // RC              # 4 chunks per branch
NSTEPS = 2 * NCH             # interleaved t/s steps
CPC = RC // P                # 16 psum cols per chunk
NCOL = RPC // P              # 64 psum cols per quantity
YF = RPC * C // P            # 640
SF = RPC * SC // P           # 128

# exp bit trick: bf16 bits of exp(x) ~= round(x*K1 + K2) as int16
K1 = 128.0 * 1.4426950408889634
K2 = 128.0 * (127.0 - 0.043)

BR = ["t", "s"]

_CACHED_NC = None
LAST_EXEC_NS = None


def _build_nc():
    import concourse.bass as bass
    import concourse.tile as tile
    from concourse import mybir
    from contextlib import ExitStack

    f32 = mybir.dt.float32
    bf16 = mybir.dt.bfloat16
    i16 = mybir.dt.int16
    fp8 = mybir.dt.float8e4
    Exp = mybir.ActivationFunctionType.Exp
    Ln = mybir.ActivationFunctionType.Ln
    add = mybir.AluOpType.add
    sub = mybir.AluOpType.subtract
    mult = mybir.AluOpType.mult
    X = mybir.AxisListType.X

    nc = bass.Bass("TRN2", debug=False)

    ins = {}
    for bn in BR:
        ins[f"ls_{bn}"] = nc.dram_tensor(f"ls_{bn}", [P, RPC], fp8,
                                         kind="ExternalInput").ap()
        ins[f"eps_{bn}"] = nc.dram_tensor(f"eps_{bn}", [P, RPC], bf16,
                                          kind="ExternalInput").ap()
        ins[f"mean_{bn}"] = nc.dram_tensor(f"mean_{bn}", [P, RPC], bf16,
                                           kind="ExternalInput").ap()
        ins[f"prior_{bn}"] = nc.dram_tensor(f"prior_{bn}", [P, RPC], bf16,
                                            kind="ExternalInput").ap()
    ins["y16"] = nc.dram_tensor("y16", [P, YF], bf16, kind="ExternalInput").ap()
    ins["ypick"] = nc.dram_tensor("ypick", [P, NCOL], f32,
                                  kind="ExternalInput").ap()
    ins["sz16"] = nc.dram_tensor("sz16", [P, SF], bf16,
                                 kind="ExternalInput").ap()
    out_d = nc.dram_tensor("out", [P, 4 * NCOL], f32, kind="ExternalOutput").ap()

    with tile.TileContext(nc) as tc, ExitStack() as ctx:
        io = ctx.enter_context(tc.tile_pool(name="io", bufs=1))
        st = ctx.enter_context(tc.tile_pool(name="st", bufs=1))
        ps = ctx.enter_context(tc.tile_pool(name="ps", bufs=1, space="PSUM"))

        out_sb = st.tile([P, 4 * NCOL], f32, tag="out")
        ones_b = st.tile([P, 1], bf16, tag="ones_b")
        nc.vector.memset(ones_b[:], 1.0)

        # PSUM accumulators: [quantity][branch] -> [P, NCOL] f32
        ps_S = [ps.tile([P, NCOL], f32, tag=f"psS{b}", name=f"psS{b}")
                for b in range(2)]
        ps_RD = [ps.tile([P, NCOL], f32, tag=f"psRD{b}", name=f"psRD{b}")
                 for b in range(2)]
        ps_PS = [ps.tile([P, NCOL], f32, tag=f"psPS{b}", name=f"psPS{b}")
                 for b in range(2)]

        # ---------------- small paths first (pipeline warm-up) -------------
        y_t = st.tile([P, YF], bf16, tag="y16")
        nc.sync.dma_start(y_t[:], ins["y16"])
        yp_t = st.tile([P, NCOL], f32, tag="ypick")
        nc.sync.dma_start(yp_t[:], ins["ypick"])
        sz_t = st.tile([P, SF], bf16, tag="sz16")
        nc.sync.dma_start(sz_t[:], ins["sz16"])

        # cross entropy: ce_row = ln(sum_c exp(y)) - y_pick
        ey_t = st.tile([P, YF], bf16, tag="ey")
        nc.scalar.activation(ey_t[:], y_t[:], Exp)
        sy_t = st.tile([P, NCOL], f32, tag="sy")
        nc.vector.tensor_reduce(
            sy_t[:], ey_t[:].rearrange("p (g c) -> p g c", c=C), X, add)
        lse_t = st.tile([P, NCOL], f32, tag="lse")
        nc.scalar.activation(lse_t[:], sy_t[:], Ln)
        nc.vector.tensor_tensor(
            out_sb[:, bass.ts(2, NCOL)], lse_t[:], yp_t[:], sub)
        nc.sync.dma_start(out_d[:, bass.ts(2, NCOL)], out_sb[:, bass.ts(2, NCOL)])

        # entropy of softmax(s_zt): ln(ssum) - (sum e*x)/ssum
        esz_t = st.tile([P, SF], bf16, tag="esz")
        nc.scalar.activation(esz_t[:], sz_t[:], Exp)
        ssum_t = st.tile([P, NCOL], f32, tag="ssum")
        nc.vector.tensor_reduce(
            ssum_t[:], esz_t[:].rearrange("p (g c) -> p g c", c=SC), X, add)
        exs_t = st.tile([P, SF], bf16, tag="exs")
        nc.vector.tensor_tensor(exs_t[:], esz_t[:], sz_t[:], mult)
        dsum_t = st.tile([P, NCOL], f32, tag="dsum")
        nc.vector.tensor_reduce(
            dsum_t[:], exs_t[:].rearrange("p (g c) -> p g c", c=SC), X, add)
        rss_t = st.tile([P, NCOL], f32, tag="rss")
        nc.vector.reciprocal(rss_t[:], ssum_t[:])
        t2_t = st.tile([P, NCOL], f32, tag="t2")
        nc.vector.tensor_tensor(t2_t[:], dsum_t[:], rss_t[:], mult)
        lss_t = st.tile([P, NCOL], f32, tag="lss")
        nc.scalar.activation(lss_t[:], ssum_t[:], Ln)
        nc.vector.tensor_tensor(
            out_sb[:, bass.ts(3, NCOL)], lss_t[:], t2_t[:], sub)
        nc.sync.dma_start(out_d[:, bass.ts(3, NCOL)], out_sb[:, bass.ts(3, NCOL)])

        # ---------------- upfront input DMAs -------------------------------
        # all big tensors fit in SBUF (112KB/partition).  Stream them in
        # chunk-granular DMAs ordered by first use (both branches' chunk 0
        # first) so the pipeline starts ~15us earlier than a slab stream.
        full = {}
        for tname, dt_ in (("ls", fp8), ("prior", bf16),
                           ("eps", bf16), ("mean", bf16)):
            for bn in BR:
                full[(bn, tname)] = st.tile(
                    [P, RPC], dt_, tag=f"{tname}_{bn}", name=f"{tname}_{bn}")
        # single SP (sync) HWDGE queue, chunk-granular in consumption
        # order.  (Measured dead ends: gpsimd SWDGE as a second queue
        # runs at half rate; scalar-engine HWDGE as a second queue is
        # slower still; half-tensor DMAs save queue handoffs but delay
        # the pipeline start more than they save.)
        for c in range(NCH):
            for bn in BR:
                for tname in ("ls", "prior", "eps", "mean"):
                    nc.sync.dma_start(
                        full[(bn, tname)][:, bass.ts(c, RC)],
                        ins[f"{tname}_{bn}"][:, bass.ts(c, RC)])

        def chunk(bn, tname, c):
            return full[(bn, tname)][:, c * RC:(c + 1) * RC]

        # ---------------- main pipeline ------------------------------------
        # step s: branch b = s % 2, chunk c = s // 2, col range c*CPC..
        # Engines: ACT: std, e.  DVE: pe bit-trick, se, enc, d, ed.
        # PE: per-row sums.  (Pool deliberately idle: its software TTs
        # stall concurrent DVE ops ~4x.)
        state = {}

        def stage1(s):
            # ACT std; pe = exp(prior): ACT for branch t, DVE bit-trick for
            # branch s (load balance)
            b, c = s % 2, s // 2
            bn = BR[b]
            t = state[s] = {}
            t["std"] = io.tile([P, RC], bf16, tag="std", name=f"std{s}", bufs=3)
            nc.scalar.activation(t["std"][:], chunk(bn, "ls", c), Exp, scale=0.5)
            t["pe"] = io.tile([P, RC], bf16, tag="pe", name=f"pe{s}", bufs=3)
            if b == 0:
                nc.scalar.activation(t["pe"][:], chunk(bn, "prior", c), Exp)
            else:
                nc.vector.tensor_scalar(
                    t["pe"][:].bitcast(i16), chunk(bn, "prior", c), K1, K2,
                    mult, add)

        def stage2(s):
            # DVE se = std * eps; PE: per-row sums of pe into PSUM
            b, c = s % 2, s // 2
            bn = BR[b]
            t = state[s]
            t["se"] = io.tile([P, RC], bf16, tag="se", name=f"se{s}", bufs=3)
            nc.vector.tensor_tensor(
                t["se"][:], t["std"][:], chunk(bn, "eps", c), mult)
            for k in range(CPC):
                nc.tensor.matmul(
                    ps_PS[b][:, c * CPC + k:c * CPC + k + 1],
                    t["pe"][:, k * P:(k + 1) * P], ones_b[:])

        def stage3(s):
            # DVE enc, d; ACT e
            b, c = s % 2, s // 2
            bn = BR[b]
            t = state[s]
            t["enc"] = io.tile([P, RC], bf16, tag="enc", name=f"enc{s}", bufs=3)
            nc.vector.tensor_tensor(
                t["enc"][:], t["se"][:], chunk(bn, "mean", c), add)
            t["d"] = io.tile([P, RC], bf16, tag="d", name=f"d{s}", bufs=3)
            nc.vector.tensor_tensor(
                t["d"][:], t["enc"][:], chunk(bn, "prior", c), sub)
            t["e"] = io.tile([P, RC], bf16, tag="e", name=f"e{s}", bufs=3)
            nc.scalar.activation(t["e"][:], t["enc"][:], Exp)

        def stage4(s):
            # DVE ed; PE: sums of e and ed
            b, c = s % 2, s // 2
            t = state.pop(s)
            t["ed"] = io.tile([P, RC], bf16, tag="ed", name=f"ed{s}", bufs=3)
            nc.vector.tensor_tensor(t["ed"][:], t["e"][:], t["d"][:], mult)
            for k in range(CPC):
                nc.tensor.matmul(
                    ps_S[b][:, c * CPC + k:c * CPC + k + 1],
                    t["e"][:, k * P:(k + 1) * P], ones_b[:])
            for k in range(CPC):
                nc.tensor.matmul(
                    ps_RD[b][:, c * CPC + k:c * CPC + k + 1],
                    t["ed"][:, k * P:(k + 1) * P], ones_b[:])

        for i in range(NSTEPS + 3):
            if i < NSTEPS:
                stage1(i)
            if 1 <= i and i - 1 < NSTEPS:
                stage2(i - 1)
            if 2 <= i and i - 2 < NSTEPS:
                stage3(i - 2)
            if 3 <= i and i - 3 < NSTEPS:
                stage4(i - 3)

        # ---------------- per-branch tails ---------------------------------
        for b in range(2):
            rs_t = st.tile([P, NCOL], f32, tag=f"rs{b}")
            nc.vector.reciprocal(rs_t[:], ps_S[b][:])
            term_t = st.tile([P, NCOL], f32, tag=f"term{b}")
            nc.vector.tensor_tensor(term_t[:], ps_RD[b][:], rs_t[:], mult)
            lnS_t = st.tile([P, NCOL], f32, tag=f"lnS{b}")
            nc.scalar.activation(lnS_t[:], ps_S[b][:], Ln)
            lnPS_t = st.tile([P, NCOL], f32, tag=f"lnPS{b}")
            nc.scalar.activation(lnPS_t[:], ps_PS[b][:], Ln)
            tmp_t = st.tile([P, NCOL], f32, tag=f"tmp{b}")
            nc.vector.tensor_tensor(tmp_t[:], term_t[:], lnS_t[:], sub)
            nc.vector.tensor_tensor(
                out_sb[:, bass.ts(b, NCOL)], tmp_t[:], lnPS_t[:], add)
            nc.sync.dma_start(
                out_d[:, bass.ts(b, NCOL)], out_sb[:, bass.ts(b, NCOL)])

    return nc


def _split_multi_waits(nc):
    """walrus's codegen allows a single embedded sync-wait per compute
    instruction; Tile sometimes emits two.  Hoist all-but-one wait into
    standalone EventSemaphore instructions on the same engine."""
    import json

    orig = nc.to_json_bytes

    def patched():
        bj = json.loads(orig())
        for fn in bj["functions"]:
            for blk in fn["blocks"]:
                new = []
                for inst in blk["instructions"]:
                    si = inst.get("sync_info") or {}
                    waits = si.get("on_wait") or []
                    if len(waits) > 1 and inst.get("opcode") != "EventSemaphore":
                        for i, w in enumerate(waits[:-1]):
                            new.append({
                                "debug": inst.get("debug"),
                                "engine": inst["engine"],
                                "ins": [],
                                "name": f"{inst['name']}-sw{i}",
                                "opcode": "EventSemaphore",
                                "outs": [],
                                "sync_info": {"on_update": [], "on_wait": [w]},
                            })
                        si["on_wait"] = [waits[-1]]
                    new.append(inst)
                blk["instructions"] = new
        return json.dumps(bj).encode()

    nc.to_json_bytes = patched
    return nc


def get_nc():
    global _CACHED_NC
    if _CACHED_NC is None:
        _CACHED_NC = _split_multi_waits(_build_nc())
    return _CACHED_NC


def make_in_maps(inputs):
    """Host-side sharding: slice rows, cast dtypes, transpose to
    [D, rows] for the big tensors, gather y_pick."""
    f8 = ml_dtypes.float8_e4m3
    bf = ml_dtypes.bfloat16
    arr = {k: np.asarray(v) for k, v in inputs.items()}
    target = np.asarray(arr["target"]).astype(np.int64).reshape(B)
    y32 = np.asarray(arr["y_zt"], np.float32)
    ypick_full = y32[np.arange(B), target]

    srcs = {
        "t": ("log_std_t", "eps_t", "mean_t", "eps_prior_t"),
        "s": ("log_std_s", "eps_s", "mean_s", "eps_prior_s"),
    }
    in_maps = []
    for cidx in range(NCORES):
        sl = slice(cidx * RPC, (cidx + 1) * RPC)
        m = {}
        for bn, (ls_k, eps_k, mean_k, prior_k) in srcs.items():
            m[f"ls_{bn}"] = np.ascontiguousarray(
                np.asarray(arr[ls_k][sl], np.float32).astype(f8).T)
            m[f"eps_{bn}"] = np.ascontiguousarray(
                np.asarray(arr[eps_k][sl], np.float32).astype(bf).T)
            m[f"mean_{bn}"] = np.ascontiguousarray(
                np.asarray(arr[mean_k][sl], np.float32).astype(bf).T)
            m[f"prior_{bn}"] = np.ascontiguousarray(
                np.asarray(arr[prior_k][sl], np.float32).astype(bf).T)
        m["y16"] = np.ascontiguousarray(
            y32[sl].astype(bf).reshape(P, YF))
        m["ypick"] = np.ascontiguousarray(
            ypick_full[sl].astype(np.float32).reshape(P, NCOL))
        m["sz16"] = np.ascontiguousarray(
            np.asarray(arr["s_zt"][sl], np.float32).astype(bf).reshape(P, SF))
        in_maps.append(m)
    return in_maps


def combine(outs, current_step):
    """Host-side unshard: f64 reduce of per-row partials -> final scalar."""
    tot = np.zeros(4, dtype=np.float64)
    for o in outs:
        o = np.asarray(o, np.float64).reshape(P, 4, NCOL)
        tot += o.sum(axis=(0, 2))
    L_zt, L_zs, L_t, Loss_e = tot / B
    frac = float(current_step) / STEP_SIZE
    lam_e = LAMBDA_E * GAMMA_E ** frac
    lam_od = LAMBDA_OD * GAMMA_OD ** frac
    val = L_t + lam_e * Loss_e + lam_od * (L_zt + L_zs)
    return np.array(val, dtype=np.float32)


def _install_ntff_hook():
    """Best-effort: register the axon NTFF profiling hook so trace=True
    yields exec_time_ns."""
    try:
        import sys, types
        import antenv
        if "antenv.axon_hooks" in sys.modules:
            return True
        sys.path.insert(0, "/root/.axon_site/trn_agent_boot")
        import trn_boot
        mod = types.ModuleType("antenv.axon_hooks")
        _h = {}
        mod.set_axon_ntff_profile_hook = lambda h: _h.__setitem__("h", h)
        mod.get_axon_ntff_profile_hook = lambda: _h.get("h")
        sys.modules["antenv.axon_hooks"] = mod
        antenv.axon_hooks = mod
        mod.set_axon_ntff_profile_hook(
            trn_boot._ntff_profile_via_ctypes("/opt/axon/libaxon_pjrt.so")
        )
        import concourse.bass_utils as bu
        bu.upload_artifacts = lambda tmpdir: str(tmpdir)
        return True
    except Exception:
        return False


def kernel(**inputs):
    global LAST_EXEC_NS
    from concourse.bass_utils import run_bass_kernel_spmd

    trace = os.environ.get("BASS_KERNEL_TRACE", "0") == "1"
    if trace:
        trace = _install_ntff_hook()

    nc = get_nc()
    in_maps = make_in_maps(inputs)
    res = run_bass_kernel_spmd(
        nc, in_maps, list(range(NCORES)), trace=trace
    )
    LAST_EXEC_NS = res.exec_time_ns
    outs = [r["out"] for r in res.results]
    cs = inputs.get("current_step", 500)
    return combine(outs, int(np.asarray(cs)))
